# revision 14
# baseline (speedup 1.0000x reference)
"""Distributed Trainium2 Bass kernel for a 4-layer GPT-style transformer.

Sharding: 8 cores = 2 batch groups x 4-way sequence parallel.
  - core c: batch element g = c//4, token slice r = c%4 (tokens
    256r..256r+255), vocab shard r for the tied LM head.
  - Per layer each core projects Q/K/V for its 256 tokens, AllGathers
    K^T and V across its group of 4, runs causal attention for its
    queries against all keys, then out-proj + FFN token-parallel.
  - Final hidden states are AllGathered so every core computes its
    vocab shard of the LM head over all 1024 tokens.

On-chip layout: residual stream transposed (features on partitions,
tokens on free).  Scores computed keys-on-partitions (k @ q^T); the
AV matmul is flipped to queries-on-partitions with a ones-column
appended to V so the softmax denominator falls out of the same
accumulation for free.  Causality = 0/1 mask multiply after exp (mask
is per-core input data, keeping the SPMD graph uniform).  Matmuls
bf16, residual stream fp32.  Weights streamed from DRAM per output
tile in stationary-tile-major layout (contiguous DMA).
"""

import numpy as np
import ml_dtypes

import concourse.bass as bass
import concourse.mybir as mybir
import concourse.tile as tile
from concourse import bacc
from concourse.bass_utils import run_bass_kernel_spmd
from concourse.masks import make_identity

V, E, NH, HD, L, T, B, FF = 50257, 768, 12, 64, 4, 1024, 2, 3072
EPS = 1e-5
P = 128
KE = E // P            # 6 feature subtiles
KF = FF // P           # 24
TL = 256               # local tokens per core
NT = T // P            # 8 key tiles
VP = 12672             # vocab shard per core (99 * 128)
MV = VP // P           # 99
RG = [[0, 1, 2, 3], [4, 5, 6, 7]]
BF16 = mybir.dt.bfloat16
F32 = mybir.dt.float32
AF = mybir.ActivationFunctionType
OP = mybir.AluOpType
BF = ml_dtypes.bfloat16

_CACHE = {}


def _build():
    nc = bacc.Bacc("TRN2", target_bir_lowering=False, debug=False,
                   num_devices=8)

    x0t = nc.declare_dram_parameter("x0t", [E, TL], F32, isOutput=False)
    wqk = nc.declare_dram_parameter("wqk", [L, 2 * KE, P, KE * P], BF16,
                                    isOutput=False)
    wv = nc.declare_dram_parameter("wv", [L, P, KE * E], BF16, isOutput=False)
    wout = nc.declare_dram_parameter("wout", [L, KE, P, KE * P], BF16,
                                     isOutput=False)
    wfc1 = nc.declare_dram_parameter("wfc1", [L, KF, P, KE * P], BF16,
                                     isOutput=False)
    bfc1 = nc.declare_dram_parameter("bfc1", [L, P, KF], F32, isOutput=False)
    wfc2 = nc.declare_dram_parameter("wfc2", [L, KE, P, KF * P], BF16,
                                     isOutput=False)
    bfc2 = nc.declare_dram_parameter("bfc2", [L, P, KE], F32, isOutput=False)
    wemb = nc.declare_dram_parameter("wemb", [MV, P, KE * P], BF16,
                                     isOutput=False)
    maskp = nc.declare_dram_parameter("mask", [NT, P, TL], BF16,
                                      isOutput=False)
    out = nc.declare_dram_parameter("out", [MV, P, T], BF16, isOutput=True)

    with tile.TileContext(nc) as tc:
        with (
            tc.tile_pool(name="resident", bufs=1) as res,
            tc.tile_pool(name="wts", bufs=2) as wpool,
            tc.tile_pool(name="wstream", bufs=3) as wst,
            tc.tile_pool(name="acts", bufs=2) as apool,
            tc.tile_pool(name="small", bufs=3) as spool,
            tc.tile_pool(name="dram", bufs=2, space="DRAM") as dpool,
            tc.tile_pool(name="ps", bufs=1, space="PSUM") as psp,
        ):
            # --- resident tiles ---
            x = res.tile([P, KE, TL], F32)         # residual stream (xT)
            xhat = res.tile([P, KE, TL], BF16)     # normalized, bf16
            mask = res.tile([P, NT, TL], BF16)     # causal masks (per-core)
            ones_c = res.tile([P, 1], BF16)
            ones_r = res.tile([1, P], F32)
            eps_c = res.tile([1, 1], F32)
            ident = res.tile([P, P], BF16)
            nr_b = res.tile([P, 2, TL], F32)   # -mean / rstd broadcast
            q_s = res.tile([P, KE, TL], BF16)      # Q^T local
            kq_l = res.tile([P, KE, TL], BF16)     # K^T local (pre-gather)
            v_l = res.tile([P, 2, E], BF16)        # V local (tok, E)
            kg = res.tile([P, KE, 4, TL], BF16)    # K^T gathered
            vg = res.tile([P, NT, NH, HD + 1], BF16)  # V gathered + ones col
            o_q = res.tile([P, 2, E], BF16)        # attn out, queries on part
            o_t = res.tile([P, KE, TL], BF16)      # attn out, transposed
            h1 = res.tile([P, KF, TL], BF16)       # FFN hidden
            xf = res.tile([P, KE, 4, TL], BF16)    # final hidden, gathered

            nc.any.memset(ones_c[:], 1.0)
            nc.any.memset(ones_r[:], 1.0)
            nc.any.memset(eps_c[:], EPS)
            nc.any.memset(vg[:, :, :, HD:HD + 1], 1.0)
            make_identity(nc, ident[:])
            nc.sync.dma_start(mask[:], maskp.ap().rearrange("n p t -> p n t"))
            nc.sync.dma_start(x[:], x0t.ap().rearrange("(ko p) t -> p ko t",
                                                       p=P))

            def layernorm():
                """x (f32) -> xhat (bf16), pure normalize (scales folded).

                Sum and sum-of-squares go to SEPARATE PSUM banks:
                interleaved multi-instruction accumulation groups sharing
                one bank corrupt each other on hardware."""
                ps_s = psp.tile([P, 2, TL], F32, tag="sc", bufs=3)
                ps_q = psp.tile([P, 2, TL], F32, tag="sc", bufs=3)
                for k in range(KE):
                    xbt = spool.tile([P, TL], BF16, tag="xbt")
                    nc.vector.tensor_copy(out=xbt[:], in_=x[:, k, :])
                    nc.tensor.matmul(ps_s[:1, 0, :], ones_c[:], xbt[:],
                                     start=(k == 0), stop=(k == KE - 1))
                    xsq = spool.tile([P, TL], BF16, tag="xsq")
                    nc.vector.tensor_tensor(xsq[:], xbt[:], xbt[:], OP.mult)
                    nc.tensor.matmul(ps_q[:1, 0, :], ones_c[:], xsq[:],
                                     start=(k == 0), stop=(k == KE - 1))
                nm = spool.tile([1, TL], F32, tag="nm")
                t_m = spool.tile([1, TL], F32, tag="t_m")
                t_v = spool.tile([1, TL], F32, tag="t_v")
                nc.vector.tensor_scalar_mul(nm, ps_s[:1, 0, :], -1.0 / E)
                nc.vector.tensor_scalar_mul(t_m, ps_s[:1, 0, :], 1.0 / E)
                nc.vector.tensor_scalar_mul(t_v, ps_q[:1, 0, :], 1.0 / E)
                nc.vector.tensor_tensor(t_m, t_m, t_m, OP.mult)
                nc.vector.tensor_tensor(t_v, t_v, t_m, OP.subtract)
                nc.scalar.activation(t_v, t_v, AF.Sqrt, bias=eps_c[:])
                nc.vector.reciprocal(t_m, t_v)
                ps_b = psp.tile([P, 512], F32, tag="mm", bufs=2)
                nc.tensor.matmul(ps_b[:, :TL], ones_r[:], nm,
                                 start=True, stop=True)
                nc.tensor.matmul(ps_b[:, TL:], ones_r[:], t_m,
                                 start=True, stop=True)
                nc.vector.tensor_copy(out=nr_b[:, 0, :], in_=ps_b[:, :TL])
                nc.vector.tensor_copy(out=nr_b[:, 1, :], in_=ps_b[:, TL:])
                for k in range(KE):
                    tmp = spool.tile([P, TL], F32, tag="lnt")
                    nc.vector.tensor_tensor(tmp, x[:, k, :], nr_b[:, 0, :],
                                            OP.add)
                    nc.vector.tensor_tensor(xhat[:, k, :], tmp, nr_b[:, 1, :],
                                            OP.mult)

            def proj6(dram_l, dst, m0):
                """6 stationary tiles of dram_l -> dst [P, KE, TL] bf16."""
                for j in range(KE):
                    wt = wst.tile([P, KE * P], BF16, tag="w6")
                    nc.sync.dma_start(wt[:], dram_l[m0 + j])
                    ps = psp.tile([P, 512], F32, tag="mm", bufs=2)
                    for k in range(KE):
                        nc.tensor.matmul(
                            ps[:, :TL], wt[:, k * P:(k + 1) * P],
                            xhat[:, k, :], start=(k == 0), stop=(k == KE - 1))
                    nc.vector.tensor_copy(out=dst[:, j, :], in_=ps[:, :TL])

            for l in range(L):
                wv_t = wpool.tile([P, KE * E], BF16, tag="wv")
                b1_t = wpool.tile([P, KF], F32, tag="b1")
                b2_t = wpool.tile([P, KE], F32, tag="b2")
                nc.sync.dma_start(wv_t[:], wv.ap()[l])
                nc.sync.dma_start(b1_t[:], bfc1.ap()[l])
                nc.sync.dma_start(b2_t[:], bfc2.ap()[l])

                layernorm()

                # ---- K and V projections, one combined AllGather ----
                # kv_i bytes: rows 0..E-1 = K^T [E, TL]; rows E..2E-1 = the
                # raw bytes of V [TL, E] (three 256-col rows per token).
                proj6(wqk.ap()[l], kq_l, KE)
                kv_i = dpool.tile([2 * E, TL], BF16, tag="kvi")
                kv_o = dpool.tile([8 * E, TL], BF16, tag="kvo")
                nc.gpsimd.dma_start(
                    kv_i[:E, :].rearrange("(ko p) t -> p ko t", p=P), kq_l[:])
                for tt in range(2):
                    for (f0, fn) in ((0, 512), (512, E - 512)):
                        ps = psp.tile([P, 512], F32, tag="mm", bufs=2)
                        for k in range(KE):
                            nc.tensor.matmul(
                                ps[:, :fn], xhat[:, k, tt * P:(tt + 1) * P],
                                wv_t[:, k * E + f0:k * E + f0 + fn],
                                start=(k == 0), stop=(k == KE - 1))
                        nc.vector.tensor_copy(
                            out=v_l[:, tt, f0:f0 + fn], in_=ps[:, :fn])
                for tt in range(2):
                    nc.gpsimd.dma_start(
                        kv_i[E + 3 * P * tt:E + 3 * P * (tt + 1), :]
                        .rearrange("(p fb) c -> p fb c", fb=3),
                        v_l[:, tt, :].rearrange("p (fb c) -> p fb c", fb=3))
                nc.gpsimd.collective_compute(
                    "AllGather", OP.bypass, replica_groups=RG,
                    ins=[kv_i.opt()], outs=[kv_o.opt()])

                # ---- Q projection (overlaps the gather) ----
                proj6(wqk.ap()[l], q_s, 0)

                # ---- land gathered K^T and V ----
                for b in range(4):
                    r0 = 2 * E * b
                    nc.gpsimd.dma_start(
                        kg[:, :, b, :],
                        kv_o[r0:r0 + E, :].rearrange(
                            "(ko p) t -> p ko t", p=P))
                    for j in range(2):
                        v0 = r0 + E + 3 * P * j
                        for fb in range(3):
                            nc.gpsimd.dma_start(
                                vg[:, 2 * b + j, 4 * fb:4 * fb + 4, 0:HD],
                                kv_o[v0:v0 + 3 * P, :].rearrange(
                                    "(p fb) c -> p fb c", fb=3)[:, fb, :])

                # ---- attention per head ----
                for h in range(NH):
                    mt, mo = divmod(h * HD, P)
                    pt = apool.tile([P, NT, TL], BF16, tag="pt")
                    for kk in range(NT // 2):
                        ps_sc = psp.tile([P, 2, TL], F32, tag="sc", bufs=3)
                        for j in range(2):
                            kt = 2 * kk + j
                            ko_b, ko_o = kt // 2, (kt % 2) * P
                            nc.tensor.matmul(
                                ps_sc[:, j, :],
                                kg[mo:mo + HD, mt, ko_b, ko_o:ko_o + P],
                                q_s[mo:mo + HD, mt, :],
                                start=True, stop=True)
                        nc.scalar.activation(
                            pt[:, 2 * kk:2 * kk + 2, :], ps_sc[:], AF.Exp)
                        nc.vector.tensor_tensor(
                            pt[:, 2 * kk:2 * kk + 2, :],
                            pt[:, 2 * kk:2 * kk + 2, :],
                            mask[:, 2 * kk:2 * kk + 2, :], OP.mult)
                    ps_av = psp.tile([P, 2, P], F32, tag="sm", bufs=2)
                    for qt in range(2):
                        for kt in range(NT):
                            nc.tensor.matmul(
                                ps_av[:, qt, :HD + 1],
                                pt[:, kt, qt * P:(qt + 1) * P],
                                vg[:, kt, h, :],
                                start=(kt == 0), stop=(kt == NT - 1))
                    for qt in range(2):
                        rd = spool.tile([P, 1], F32, tag="rd")
                        nc.vector.reciprocal(rd, ps_av[:, qt, HD:HD + 1])
                        nc.vector.tensor_scalar_mul(
                            o_q[:, qt, h * HD:(h + 1) * HD],
                            ps_av[:, qt, :HD], rd)

                # ---- transpose attn output to feature-major ----
                for qt in range(2):
                    ps_t = psp.tile([P, KE, P], BF16, tag="tr", bufs=1)
                    for ko in range(KE):
                        nc.tensor.transpose(
                            ps_t[:, ko, :], o_q[:, qt, ko * P:(ko + 1) * P],
                            ident[:])
                    nc.vector.tensor_copy(
                        out=o_t[:, :, qt * P:(qt + 1) * P], in_=ps_t[:])

                # ---- output projection + residual ----
                for m in range(KE):
                    wt = wst.tile([P, KE * P], BF16, tag="w6")
                    nc.sync.dma_start(wt[:], wout.ap()[l, m])
                    ps = psp.tile([P, 512], F32, tag="mm", bufs=2)
                    for k in range(KE):
                        nc.tensor.matmul(
                            ps[:, :TL], wt[:, k * P:(k + 1) * P], o_t[:, k, :],
                            start=(k == 0), stop=(k == KE - 1))
                    nc.vector.tensor_tensor(
                        x[:, m, :], ps[:, :TL], x[:, m, :], OP.add)

                layernorm()

                # ---- FFN ----
                for m in range(KF):
                    wt = wst.tile([P, KE * P], BF16, tag="w6")
                    nc.sync.dma_start(wt[:], wfc1.ap()[l, m])
                    ps = psp.tile([P, 512], F32, tag="mm", bufs=2)
                    for k in range(KE):
                        nc.tensor.matmul(
                            ps[:, :TL], wt[:, k * P:(k + 1) * P],
                            xhat[:, k, :], start=(k == 0), stop=(k == KE - 1))
                    nc.scalar.activation(
                        h1[:, m, :], ps[:, :TL], AF.Gelu,
                        bias=b1_t[:, m:m + 1])
                for m in range(KE):
                    wt24 = wst.tile([P, KF * P], BF16, tag="w24")
                    nc.sync.dma_start(wt24[:], wfc2.ap()[l, m])
                    ps = psp.tile([P, 512], F32, tag="mm", bufs=2)
                    for k in range(KF):
                        nc.tensor.matmul(
                            ps[:, :TL], wt24[:, k * P:(k + 1) * P],
                            h1[:, k, :], start=(k == 0), stop=(k == KF - 1))
                    tmp = spool.tile([P, TL], F32, tag="f2t")
                    nc.vector.tensor_scalar_add(tmp, ps[:, :TL],
                                                b2_t[:, m:m + 1])
                    nc.vector.tensor_tensor(
                        x[:, m, :], tmp, x[:, m, :], OP.add)

            # ---- final LN + gather + LM head ----
            layernorm()
            xf_i = dpool.tile([E, TL], BF16, tag="kbi")
            xf_o = dpool.tile([4 * E, TL], BF16, tag="kbo")
            nc.gpsimd.dma_start(
                xf_i.rearrange("(ko p) t -> p ko t", p=P), xhat[:])
            nc.gpsimd.collective_compute(
                "AllGather", OP.bypass, replica_groups=RG,
                ins=[xf_i.opt()], outs=[xf_o.opt()])
            for b in range(4):
                nc.gpsimd.dma_start(
                    xf[:, :, b, :],
                    xf_o[b * E:(b + 1) * E, :].rearrange(
                        "(ko p) t -> p ko t", p=P))
            for m in range(MV):
                we = wst.tile([P, KE * P], BF16, tag="we")
                nc.sync.dma_start(we[:], wemb.ap()[m])
                for c in range(2):
                    cs = slice(c * 512, (c + 1) * 512)
                    ps = psp.tile([P, 512], F32, tag="mm", bufs=2)
                    for k in range(KE):
                        nc.tensor.matmul(
                            ps, we[:, k * P:(k + 1) * P],
                            xf[:, k, 2 * c:2 * c + 2, :],
                            start=(k == 0), stop=(k == KE - 1))
                    ot = apool.tile([P, 512], BF16, tag="ot")
                    nc.scalar.copy(ot, ps)
                    nc.sync.dma_start(out.ap()[m][:, cs], ot)

    nc.compile()
    return nc


def _prep(inputs):
    """Host-side: fold LN scales into weights, build per-core input maps."""
    ids = np.asarray(inputs["input_ids"]).astype(np.int64)
    tok = np.asarray(inputs["tok_emb"], np.float32)
    pos = np.asarray(inputs["pos_emb"], np.float32)
    qkv = np.asarray(inputs["qkv_w"], np.float32)
    ow = np.asarray(inputs["out_w"], np.float32)
    f1 = np.asarray(inputs["fc1_w"], np.float32)
    b1 = np.asarray(inputs["fc1_b"], np.float32)
    f2 = np.asarray(inputs["fc2_w"], np.float32)
    b2 = np.asarray(inputs["fc2_b"], np.float32)
    s1 = np.asarray(inputs["ln1_scale"], np.float32)
    bb1 = np.asarray(inputs["ln1_bias"], np.float32)
    s2 = np.asarray(inputs["ln2_scale"], np.float32)
    bb2 = np.asarray(inputs["ln2_bias"], np.float32)
    sf = np.asarray(inputs["lnf_scale"], np.float32)
    bf_ = np.asarray(inputs["lnf_bias"], np.float32)
    # LN biases must be zero for the fold used here (true for this model).
    assert abs(bb1).max() == 0 and abs(bb2).max() == 0 and abs(bf_).max() == 0

    x0 = tok[ids] + pos[None, :, :]                      # (B, T, E)
    x0t = np.ascontiguousarray(x0.transpose(0, 2, 1))    # (B, E, T)

    scale = HD ** -0.5
    wqk_h = np.empty((L, 2 * KE, P, KE * P), BF)
    wv_h = np.empty((L, P, KE * E), BF)
    wo_h = np.empty((L, KE, P, KE * P), BF)
    w1_h = np.empty((L, KF, P, KE * P), BF)
    w2_h = np.empty((L, KE, P, KF * P), BF)
    b1_h = np.zeros((L, P, KF), np.float32)
    b2_h = np.zeros((L, P, KE), np.float32)

    def tiles(w, nm):
        # w: [E_in, n*P] (contract rows, out cols) -> [n, P, (E_in/P)*P]
        ki = w.shape[0] // P
        return np.ascontiguousarray(
            w.reshape(ki, P, nm, P).transpose(2, 1, 0, 3).reshape(
                nm, P, ki * P))

    for l in range(L):
        wq = (qkv[l, :E] * s1[l][None, :]).T * scale
        wk = (qkv[l, E:2 * E] * s1[l][None, :]).T
        wqk_h[l] = tiles(np.concatenate([wq, wk], axis=1), 2 * KE).astype(BF)
        wv_ = (qkv[l, 2 * E:] * s1[l][None, :]).T          # [E, E]
        wv_h[l] = wv_.reshape(KE, P, E).transpose(1, 0, 2).reshape(
            P, KE * E).astype(BF)
        wo_h[l] = tiles(ow[l].T, KE).astype(BF)
        w1_h[l] = tiles((f1[l] * s2[l][None, :]).T, KF).astype(BF)
        w2_h[l] = tiles(f2[l].T, KE).astype(BF)
        b1_h[l] = b1[l].reshape(KF, P).T
        b2_h[l] = b2[l].reshape(KE, P).T

    tokp = np.zeros((4 * VP, E), np.float32)
    tokp[:V] = tok * sf[None, :]
    emb_h = [tiles(np.ascontiguousarray(tokp[j * VP:(j + 1) * VP].T), MV)
             .astype(BF) for j in range(4)]

    # causal masks per token-slice r: key kt*128+kk visible to query
    # r*256+qq iff key <= query
    mask_h = []
    for r in range(4):
        gk = np.arange(NT * P)[:, None]
        gq = r * TL + np.arange(TL)[None, :]
        mask_h.append((gk <= gq).reshape(NT, P, TL).astype(BF))

    in_maps = []
    for c in range(8):
        g, r = c // 4, c % 4
        in_maps.append({
            "x0t": np.ascontiguousarray(x0t[g][:, r * TL:(r + 1) * TL]),
            "wqk": wqk_h, "wv": wv_h, "wout": wo_h,
            "wfc1": w1_h, "bfc1": b1_h, "wfc2": w2_h, "bfc2": b2_h,
            "wemb": emb_h[r], "mask": mask_h[r],
        })
    return in_maps


def kernel(**inputs) -> np.ndarray:
    if "nc" not in _CACHE:
        _CACHE["nc"] = _build()
    nc = _CACHE["nc"]
    in_maps = _prep(inputs)
    res = run_bass_kernel_spmd(nc, in_maps, list(range(8)),
                               **_CACHE.get("run_kwargs", {}))
    _CACHE["last"] = res
    logits = np.empty((B, T, V), np.float32)
    for c in range(8):
        g, r = c // 4, c % 4
        lo = r * VP
        hi = min(V, lo + VP)
        shard = res.results[c]["out"].reshape(VP, T).astype(np.float32)
        logits[g, :, lo:hi] = shard[:hi - lo].T
    return logits
